# revision 18
# baseline (speedup 1.0000x reference)
"""Trainium2 Bass kernel for nn_ChannelMerger.

Computation (per batch b):
    emb   = fourier_emb(positions[b])            # [C, 288]
    scores= emb @ heads.T                        # [C, O] (transposed layout on device)
    w     = softmax(scores over C)
    out[b]= w.T @ meg[b]                         # [O, T]

Sharding: data-parallel over batch B=32 across 8 cores (4 batches/core).
heads (tiny) replicated. Everything computed on-device; host only reshapes
inputs (transpose positions/heads, constant table) and gathers outputs.

Device layout notes:
  - emb is built transposed ([d, c], d on partitions) so it can feed the
    scores matmul directly as the stationary operand.
  - softmax runs un-max-subtracted (scores are O(4), exp is safe in fp32);
    the 1/sum is folded into the PSUM->SBUF eviction of the PV matmul as a
    per-partition scale.
  - d-dimension is permuted (cos 0:128 | sin 0:128 | cos 128:144, sin
    128:144) so each ACT sin/cos call has matching in/out partition bases;
    heads rows are permuted identically on the host.
  - ACT's Sin is only valid on [-pi, pi]; arguments are range-reduced on DVE
    via an int32 cast (r = x - 2pi*int(x/2pi), one is_ge correction) since
    the HW has no mod/floor ALU op. The +pi/2 cos shift rides the loc matmul
    as a third contraction row.
"""

import math

import numpy as np

import concourse.bass as bass
import concourse.mybir as mybir
import concourse.tile as tile
from concourse import bacc

F32 = mybir.dt.float32

B, C, T = 32, 273, 8192
O, D = 270, 288
N_CORES = 8
BPC = B // N_CORES  # batches per core
MARGIN = 0.2
N_FREQ = 12  # 12 freqs/axis; D = 2 * 12 * 12
TWO_PI = 2.0 * math.pi
HALF_PI = 0.5 * math.pi

TS = 2048  # T super-tile (per-DMA free size)
NSL = TS // 512  # 512-wide matmul slices per super-tile

C_CHUNKS = [(0, 128), (128, 128), (256, C - 256)]  # contraction over channels
O_CHUNKS = [(0, 128), (128, 128), (256, O - 256)]  # output-channel chunks
K_CHUNKS = [(0, 128), (128, 128), (256, 32)]  # device-d (permuted emb dim) chunks

# device-d row r <-> original emb dim perm[r] (see embT construction below)
PERM = (
    list(range(0, 128))  # cos(loc[0:128])
    + list(range(144, 272))  # sin(loc[0:128])
    + list(range(128, 144))  # cos(loc[128:144])
    + list(range(272, 288))  # sin(loc[128:144])
)

_SIN = mybir.ActivationFunctionType.Sin
_EXP = mybir.ActivationFunctionType.Exp


def _build_module() -> bass.Bass:
    # Bacc (not bare Bass): its compile() splits multi-sem waits — TRN2
    # instructions carry at most one wait condition and walrus rejects more.
    nc = bacc.Bacc()
    meg_h = nc.dram_tensor("meg", [BPC, C, T], F32, kind="ExternalInput")
    posT_h = nc.dram_tensor("posT", [BPC, 3, C], F32, kind="ExternalInput")
    headsTp_h = nc.dram_tensor("headsTp", [D, O], F32, kind="ExternalInput")
    pconst_h = nc.dram_tensor("pconst", [3, D], F32, kind="ExternalInput")
    out_h = nc.dram_tensor("out", [BPC, O, T], F32, kind="ExternalOutput")

    with tile.TileContext(nc) as tc:
        with (
            tc.tile_pool(name="const", bufs=1) as const,
            tc.tile_pool(name="small", bufs=2) as small,
            tc.tile_pool(name="megp", bufs=3) as megp,
            tc.tile_pool(name="outp", bufs=4) as outp,
            tc.tile_pool(name="ps_small", bufs=1, space="PSUM") as ps_small,
            tc.tile_pool(name="ps_pv", bufs=4, space="PSUM") as ps_pv,
        ):
            # ---- persistent constants ----
            hT = []
            for ki, (k0, ksz) in enumerate(K_CHUNKS):
                t_ = const.tile([ksz, O], F32, tag=f"hT{ki}", name=f"hT{ki}")
                nc.sync.dma_start(out=t_, in_=headsTp_h[k0 : k0 + ksz, :])
                hT.append(t_)
            pconst_sb = const.tile([3, D], F32, tag="pconst", name="pconst_sb")
            nc.sync.dma_start(out=pconst_sb, in_=pconst_h[:, :])
            ones_c = const.tile([128, 1], F32, tag="ones", name="ones_c")
            nc.vector.memset(ones_c, 1.0)

            for b in range(BPC):
                # ---- fourier embedding (transposed: [d, c]) ----
                # posT rows: [x + margin, y + margin, 1]; pconst cols carry
                # (p_i, p_j, shift) so loc [+ pi/2 for cos] comes out of one
                # K=3 matmul per tile.
                posT_sb = small.tile([3, C], F32, tag="posT", name="posT_sb")
                nc.sync.dma_start(out=posT_sb, in_=posT_h[b])

                # pconst col r = (p_i, p_j, shift) for device-d row r, so each
                # K-chunk of the permuted embT comes from one matmul, one DVE
                # range-reduction, and one ACT Sin — all at partition base 0.
                embs = []
                for ki, (k0, ksz) in enumerate(K_CHUNKS):
                    loc_ps = ps_small.tile(
                        [128, C], F32, tag="loc", name="loc_ps", bufs=2
                    )[:ksz]
                    nc.tensor.matmul(
                        loc_ps,
                        pconst_sb[:, k0 : k0 + ksz],
                        posT_sb,
                        start=True,
                        stop=True,
                    )
                    # ACT Sin is valid on [-pi, pi] only and the HW has no
                    # mod/floor op: reduce with r = x - 2pi*int(x * 1/2pi)
                    # (x >= 0 here). Whether the int cast truncates or rounds
                    # to nearest, r lands in [-pi, 2pi); one is_ge(pi)
                    # correction of -2pi brings it into [-pi, pi).
                    n_i = small.tile(
                        [128, C], mybir.dt.int32, tag="n_i", name="n_i", bufs=3
                    )[:ksz]
                    nc.vector.tensor_scalar(
                        out=n_i,
                        in0=loc_ps,
                        scalar1=1.0 / TWO_PI,
                        scalar2=None,
                        op0=mybir.AluOpType.mult,
                    )
                    n_f = small.tile([128, C], F32, tag="n_f", name="n_f", bufs=3)[
                        :ksz
                    ]
                    nc.vector.tensor_copy(n_f, n_i)
                    nc.vector.tensor_scalar(
                        out=n_f,
                        in0=n_f,
                        scalar1=-TWO_PI,
                        scalar2=None,
                        op0=mybir.AluOpType.mult,
                    )
                    r_ = small.tile([128, C], F32, tag="r_", name="r_", bufs=3)[:ksz]
                    nc.vector.tensor_add(r_, loc_ps, n_f)
                    m_ = small.tile([128, C], F32, tag="m_", name="m_", bufs=3)[:ksz]
                    nc.vector.tensor_scalar(
                        out=m_,
                        in0=r_,
                        scalar1=math.pi,
                        scalar2=-TWO_PI,
                        op0=mybir.AluOpType.is_ge,
                        op1=mybir.AluOpType.mult,
                    )
                    nc.vector.tensor_add(r_, r_, m_)
                    e_ = small.tile([128, C], F32, tag=f"embT{ki}", name=f"embT{ki}")[
                        :ksz
                    ]
                    nc.scalar.activation(e_, r_, _SIN)
                    embs.append(e_)

                # ---- scores^T = embT.T-contract with headsTp, then exp ----
                expT = []
                for ci, (c0, csz) in enumerate(C_CHUNKS):
                    sc_ps = ps_small.tile([128, O], F32, tag="sc", name="sc_ps")[:csz]
                    for ki in range(3):
                        nc.tensor.matmul(
                            sc_ps,
                            embs[ki][:, c0 : c0 + csz],
                            hT[ki],
                            start=(ki == 0),
                            stop=(ki == 2),
                        )
                    e_ = small.tile([128, O], F32, tag=f"expT{ci}", name=f"expT{ci}")[
                        :csz
                    ]
                    nc.scalar.activation(e_, sc_ps, _EXP)
                    expT.append(e_)

                # ---- softmax denominators (per o), as per-partition vectors ----
                invs = []
                for oi, (o0, osz) in enumerate(O_CHUNKS):
                    sum_ps = ps_small.tile([128, 1], F32, tag="sum", name="sum_ps")[
                        :osz
                    ]
                    for ci, (c0, csz) in enumerate(C_CHUNKS):
                        nc.tensor.matmul(
                            sum_ps,
                            expT[ci][:, o0 : o0 + osz],
                            ones_c[:csz],
                            start=(ci == 0),
                            stop=(ci == 2),
                        )
                    iv = small.tile([128, 1], F32, tag=f"inv{oi}", name=f"inv{oi}")[
                        :osz
                    ]
                    nc.vector.reciprocal(iv, sum_ps)
                    invs.append(iv)

                # ---- PV: out[b, o, t] = invsum[o] * sum_c expT[c, o] meg[c, t] ----
                for ts in range(T // TS):
                    t0 = ts * TS
                    megs = []
                    for ci, (c0, csz) in enumerate(C_CHUNKS):
                        m_ = megp.tile([csz, TS], F32, tag=f"meg{ci}", name=f"meg{ci}")
                        nc.sync.dma_start(
                            out=m_, in_=meg_h[b, c0 : c0 + csz, t0 : t0 + TS]
                        )
                        megs.append(m_)
                    for oi, (o0, osz) in enumerate(O_CHUNKS):
                        ostage = outp.tile([128, TS], F32, tag="ostage", name="ostage")[
                            :osz
                        ]
                        for sl in range(NSL):
                            pv_ps = ps_pv.tile([128, 512], F32, tag="pv", name="pv_ps")[
                                :osz
                            ]
                            for ci in range(3):
                                nc.tensor.matmul(
                                    pv_ps,
                                    expT[ci][:, o0 : o0 + osz],
                                    megs[ci][:, sl * 512 : (sl + 1) * 512],
                                    start=(ci == 0),
                                    stop=(ci == 2),
                                )
                            dst = ostage[:, sl * 512 : (sl + 1) * 512]
                            # alternate eviction engine to split the load
                            if (oi * NSL + sl) % 2 == 0:
                                nc.scalar.mul(dst, pv_ps, mul=invs[oi])
                            else:
                                nc.vector.tensor_scalar_mul(dst, pv_ps, invs[oi])
                        nc.sync.dma_start(
                            out=out_h[b, o0 : o0 + osz, t0 : t0 + TS], in_=ostage
                        )
    nc.compile()
    return nc


_MODULE_CACHE: list = []


def _get_module() -> bass.Bass:
    if not _MODULE_CACHE:
        _MODULE_CACHE.append(_build_module())
    return _MODULE_CACHE[0]


def _host_prep(meg, positions, heads):
    """Shard + lay out inputs for the 8 cores."""
    freqs = (TWO_PI / (1.0 + 2.0 * MARGIN)) * np.arange(N_FREQ, dtype=np.float64)
    # pconst col r = (p_{L//12}, p_{L%12}, shift) for device-d row r, where
    # emb dim PERM[r] is cos(loc[L]) (shift pi/2) or sin(loc[L]) (shift 0).
    emb_dim = np.array(PERM)
    is_cos = emb_dim < 144
    L = np.where(is_cos, emb_dim, emb_dim - 144)
    pconst = np.stack(
        [freqs[L // N_FREQ], freqs[L % N_FREQ], np.where(is_cos, HALF_PI, 0.0)]
    ).astype(np.float32)  # [3, 288]

    headsTp = np.ascontiguousarray(heads[:, PERM].T).astype(np.float32)  # [288, 270]

    in_maps = []
    for k in range(N_CORES):
        sl = slice(k * BPC, (k + 1) * BPC)
        posT = np.concatenate(
            [
                positions[sl].transpose(0, 2, 1) + np.float32(MARGIN),
                np.ones((BPC, 1, C), dtype=np.float32),
            ],
            axis=1,
        )
        posT = np.ascontiguousarray(posT)
        in_maps.append(
            {
                "meg": np.ascontiguousarray(meg[sl]),
                "posT": posT,
                "headsTp": headsTp,
                "pconst": pconst,
            }
        )
    return in_maps


LAST_RESULTS = None  # BassKernelResults of the most recent kernel() call


def kernel(meg: np.ndarray, positions: np.ndarray, heads: np.ndarray) -> np.ndarray:
    global LAST_RESULTS
    from concourse.bass_utils import run_bass_kernel_spmd

    nc = _get_module()
    in_maps = _host_prep(
        np.asarray(meg, dtype=np.float32),
        np.asarray(positions, dtype=np.float32),
        np.asarray(heads, dtype=np.float32),
    )
    res = run_bass_kernel_spmd(nc, in_maps, core_ids=list(range(N_CORES)))
    LAST_RESULTS = res
    return np.concatenate([r["out"] for r in res.results], axis=0)


# revision 29
# speedup vs baseline: 1.1446x; 1.1446x over previous
"""Trainium2 Bass kernel for nn_ChannelMerger.

Computation (per batch b):
    emb   = fourier_emb(positions[b])            # [C, 288]
    scores= emb @ heads.T                        # [C, O] (transposed layout on device)
    w     = softmax(scores over C)
    out[b]= w.T @ meg[b]                         # [O, T]

Sharding: data-parallel over batch B=32 across 8 cores (4 batches/core).
heads (tiny) replicated. Everything computed on-device; host only reshapes
inputs (transpose positions/heads, constant table) and gathers outputs.

Device layout notes:
  - emb is built transposed ([d, c], d on partitions) so it can feed the
    scores matmul directly as the stationary operand.
  - softmax runs un-max-subtracted (scores are O(4), exp is safe in fp32);
    the 1/sum is folded into the PSUM->SBUF eviction of the PV matmul as a
    per-partition scale.
  - d-dimension is permuted (cos 0:128 | sin 0:128 | cos 128:144, sin
    128:144) so each ACT sin/cos call has matching in/out partition bases;
    heads rows are permuted identically on the host.
  - ACT's Sin is only valid on [-pi, pi]; arguments are range-reduced on DVE
    via an int32 cast (r = x - 2pi*int(x/2pi), one is_ge correction) since
    the HW has no mod/floor ALU op. The +pi/2 cos shift rides the loc matmul
    as a third contraction row.
"""

import math

import numpy as np

import concourse.bass as bass
import concourse.mybir as mybir
import concourse.tile as tile
from concourse import bacc

F32 = mybir.dt.float32
F32R = mybir.dt.float32r  # single-pass PE matmul (fp32 is 2-pass/4x slower)

B, C, T = 32, 273, 8192
O, D = 270, 288
N_CORES = 8
BPC = B // N_CORES  # batches per core
MARGIN = 0.2
N_FREQ = 12  # 12 freqs/axis; D = 2 * 12 * 12
TWO_PI = 2.0 * math.pi
HALF_PI = 0.5 * math.pi

TS = 2048  # T super-tile (per-DMA free size)
NSL = TS // 512  # 512-wide matmul slices per super-tile

C_CHUNKS = [(0, 128), (128, 128), (256, C - 256)]  # contraction over channels
O_CHUNKS = [(0, 128), (128, 128), (256, O - 256)]  # output-channel chunks
K_CHUNKS = [(0, 128), (128, 128), (256, 32)]  # device-d (permuted emb dim) chunks

# device-d row r <-> original emb dim perm[r] (see embT construction below)
PERM = (
    list(range(0, 128))  # cos(loc[0:128])
    + list(range(144, 272))  # sin(loc[0:128])
    + list(range(128, 144))  # cos(loc[128:144])
    + list(range(272, 288))  # sin(loc[128:144])
)

_SIN = mybir.ActivationFunctionType.Sin
_EXP = mybir.ActivationFunctionType.Exp


def _build_module() -> bass.Bass:
    # Bacc (not bare Bass): its compile() splits multi-sem waits — TRN2
    # instructions carry at most one wait condition and walrus rejects more.
    nc = bacc.Bacc()
    # meg is declared float32r (same 4-byte payload) so the PV matmul runs
    # single-pass; measured precision ~1.5e-4 vs 2-pass fp32.
    meg_h = nc.dram_tensor("meg", [BPC, C, T], F32R, kind="ExternalInput")
    posT_h = nc.dram_tensor("posT", [BPC, 3, C], F32, kind="ExternalInput")
    headsTp_h = nc.dram_tensor("headsTp", [D, O], F32, kind="ExternalInput")
    pconst_h = nc.dram_tensor("pconst", [3, D], F32, kind="ExternalInput")
    out_h = nc.dram_tensor("out", [BPC, O, T], F32, kind="ExternalOutput")

    with tile.TileContext(nc) as tc:
        with (
            tc.tile_pool(name="const", bufs=1) as const,
            tc.tile_pool(name="small", bufs=2) as small,
            tc.tile_pool(name="megp", bufs=4) as megp,
            tc.tile_pool(name="outp", bufs=4) as outp,
            tc.tile_pool(name="ps_small", bufs=1, space="PSUM") as ps_small,
            tc.tile_pool(name="ps_pv", bufs=4, space="PSUM") as ps_pv,
        ):
            # ---- persistent constants ----
            hT = []
            for ki, (k0, ksz) in enumerate(K_CHUNKS):
                t_ = const.tile([ksz, O], F32, tag=f"hT{ki}", name=f"hT{ki}")
                nc.sync.dma_start(out=t_, in_=headsTp_h[k0 : k0 + ksz, :])
                hT.append(t_)
            pconst_sb = const.tile([3, D], F32, tag="pconst", name="pconst_sb")
            nc.sync.dma_start(out=pconst_sb, in_=pconst_h[:, :])
            ones_c = const.tile([128, 1], F32, tag="ones", name="ones_c")
            nc.vector.memset(ones_c, 1.0)

            for b in range(BPC):
                # ---- fourier embedding (transposed: [d, c]) ----
                # posT rows: [x + margin, y + margin, 1]; pconst cols carry
                # (p_i, p_j, shift) so loc [+ pi/2 for cos] comes out of one
                # K=3 matmul per tile.
                posT_sb = small.tile([3, C], F32, tag="posT", name="posT_sb")
                nc.sync.dma_start(out=posT_sb, in_=posT_h[b])

                # pconst col r = (p_i, p_j, shift) for device-d row r, so each
                # K-chunk of the permuted embT comes from one matmul, one DVE
                # range-reduction, and one ACT Sin — all at partition base 0.
                embs = []
                for ki, (k0, ksz) in enumerate(K_CHUNKS):
                    loc_ps = ps_small.tile(
                        [128, C], F32, tag="loc", name="loc_ps", bufs=2
                    )[:ksz]
                    nc.tensor.matmul(
                        loc_ps,
                        pconst_sb[:, k0 : k0 + ksz],
                        posT_sb,
                        start=True,
                        stop=True,
                    )
                    # ACT Sin is valid on [-pi, pi] only and the HW has no
                    # mod/floor op: reduce with r = x - 2pi*int(x * 1/2pi)
                    # (x >= 0 here). Whether the int cast truncates or rounds
                    # to nearest, r lands in [-pi, 2pi); one is_ge(pi)
                    # correction of -2pi brings it into [-pi, pi).
                    n_i = small.tile(
                        [128, C], mybir.dt.int32, tag="n_i", name="n_i", bufs=3
                    )[:ksz]
                    nc.vector.tensor_scalar(
                        out=n_i,
                        in0=loc_ps,
                        scalar1=1.0 / TWO_PI,
                        scalar2=None,
                        op0=mybir.AluOpType.mult,
                    )
                    n_f = small.tile([128, C], F32, tag="n_f", name="n_f", bufs=3)[
                        :ksz
                    ]
                    nc.vector.tensor_copy(n_f, n_i)
                    nc.vector.tensor_scalar(
                        out=n_f,
                        in0=n_f,
                        scalar1=-TWO_PI,
                        scalar2=None,
                        op0=mybir.AluOpType.mult,
                    )
                    r_ = small.tile([128, C], F32, tag="r_", name="r_", bufs=3)[:ksz]
                    nc.vector.tensor_add(r_, loc_ps, n_f)
                    m_ = small.tile([128, C], F32, tag="m_", name="m_", bufs=3)[:ksz]
                    nc.vector.tensor_scalar(
                        out=m_,
                        in0=r_,
                        scalar1=math.pi,
                        scalar2=-TWO_PI,
                        op0=mybir.AluOpType.is_ge,
                        op1=mybir.AluOpType.mult,
                    )
                    nc.vector.tensor_add(r_, r_, m_)
                    e_ = small.tile([128, C], F32, tag=f"embT{ki}", name=f"embT{ki}")[
                        :ksz
                    ]
                    nc.scalar.activation(e_, r_, _SIN)
                    embs.append(e_)

                # ---- scores^T = embT.T-contract with headsTp, then exp ----
                expT = []
                for ci, (c0, csz) in enumerate(C_CHUNKS):
                    sc_ps = ps_small.tile([128, O], F32, tag="sc", name="sc_ps")[:csz]
                    for ki in range(3):
                        nc.tensor.matmul(
                            sc_ps,
                            embs[ki][:, c0 : c0 + csz],
                            hT[ki],
                            start=(ki == 0),
                            stop=(ki == 2),
                        )
                    e_ = small.tile([128, O], F32, tag=f"expT{ci}", name=f"expT{ci}")[
                        :csz
                    ]
                    nc.scalar.activation(e_, sc_ps, _EXP)
                    expT.append(e_)

                # ---- softmax denominators (per o), as per-partition vectors ----
                invs = []
                for oi, (o0, osz) in enumerate(O_CHUNKS):
                    sum_ps = ps_small.tile([128, 1], F32, tag="sum", name="sum_ps")[
                        :osz
                    ]
                    for ci, (c0, csz) in enumerate(C_CHUNKS):
                        nc.tensor.matmul(
                            sum_ps,
                            expT[ci][:, o0 : o0 + osz],
                            ones_c[:csz],
                            start=(ci == 0),
                            stop=(ci == 2),
                        )
                    iv = small.tile([128, 1], F32, tag=f"inv{oi}", name=f"inv{oi}")[
                        :osz
                    ]
                    nc.vector.reciprocal(iv, sum_ps)
                    invs.append(iv)

                # f32r twin of expT for the single-pass PV matmul (the N=1
                # sums matmul above cannot run in f32r)
                expR = []
                for ci, (c0, csz) in enumerate(C_CHUNKS):
                    er = small.tile([128, O], F32R, tag=f"expR{ci}", name=f"expR{ci}")[
                        :csz
                    ]
                    nc.vector.tensor_copy(er, expT[ci])
                    expR.append(er)

                # ---- PV: out[b, o, t] = invsum[o] * sum_c expT[c, o] meg[c, t] ----
                for ts in range(T // TS):
                    t0 = ts * TS
                    megs = []
                    for ci, (c0, csz) in enumerate(C_CHUNKS):
                        m_ = megp.tile([csz, TS], F32R, tag=f"meg{ci}", name=f"meg{ci}")
                        nc.sync.dma_start(
                            out=m_, in_=meg_h[b, c0 : c0 + csz, t0 : t0 + TS]
                        )
                        megs.append(m_)
                    for oi, (o0, osz) in enumerate(O_CHUNKS):
                        ostage = outp.tile([128, TS], F32, tag="ostage", name="ostage")[
                            :osz
                        ]
                        for sl in range(NSL):
                            pv_ps = ps_pv.tile([128, 512], F32, tag="pv", name="pv_ps")[
                                :osz
                            ]
                            for ci in range(3):
                                nc.tensor.matmul(
                                    pv_ps,
                                    expR[ci][:, o0 : o0 + osz],
                                    megs[ci][:, sl * 512 : (sl + 1) * 512],
                                    start=(ci == 0),
                                    stop=(ci == 2),
                                )
                            dst = ostage[:, sl * 512 : (sl + 1) * 512]
                            # alternate eviction engine to split the load
                            if (oi * NSL + sl) % 2 == 0:
                                nc.scalar.mul(dst, pv_ps, mul=invs[oi])
                            else:
                                nc.vector.tensor_scalar_mul(dst, pv_ps, invs[oi])
                        nc.sync.dma_start(
                            out=out_h[b, o0 : o0 + osz, t0 : t0 + TS], in_=ostage
                        )
    nc.compile()
    return nc


_MODULE_CACHE: list = []


def _get_module() -> bass.Bass:
    if not _MODULE_CACHE:
        _MODULE_CACHE.append(_build_module())
    return _MODULE_CACHE[0]


def _host_prep(meg, positions, heads):
    """Shard + lay out inputs for the 8 cores."""
    freqs = (TWO_PI / (1.0 + 2.0 * MARGIN)) * np.arange(N_FREQ, dtype=np.float64)
    # pconst col r = (p_{L//12}, p_{L%12}, shift) for device-d row r, where
    # emb dim PERM[r] is cos(loc[L]) (shift pi/2) or sin(loc[L]) (shift 0).
    emb_dim = np.array(PERM)
    is_cos = emb_dim < 144
    L = np.where(is_cos, emb_dim, emb_dim - 144)
    pconst = np.stack(
        [freqs[L // N_FREQ], freqs[L % N_FREQ], np.where(is_cos, HALF_PI, 0.0)]
    ).astype(np.float32)  # [3, 288]

    headsTp = np.ascontiguousarray(heads[:, PERM].T).astype(np.float32)  # [288, 270]

    in_maps = []
    for k in range(N_CORES):
        sl = slice(k * BPC, (k + 1) * BPC)
        posT = np.concatenate(
            [
                positions[sl].transpose(0, 2, 1) + np.float32(MARGIN),
                np.ones((BPC, 1, C), dtype=np.float32),
            ],
            axis=1,
        )
        posT = np.ascontiguousarray(posT)
        in_maps.append(
            {
                "meg": np.ascontiguousarray(meg[sl]),
                "posT": posT,
                "headsTp": headsTp,
                "pconst": pconst,
            }
        )
    return in_maps


LAST_RESULTS = None  # BassKernelResults of the most recent kernel() call


def kernel(meg: np.ndarray, positions: np.ndarray, heads: np.ndarray) -> np.ndarray:
    global LAST_RESULTS
    from concourse.bass_utils import run_bass_kernel_spmd

    nc = _get_module()
    in_maps = _host_prep(
        np.asarray(meg, dtype=np.float32),
        np.asarray(positions, dtype=np.float32),
        np.asarray(heads, dtype=np.float32),
    )
    res = run_bass_kernel_spmd(nc, in_maps, core_ids=list(range(N_CORES)))
    LAST_RESULTS = res
    return np.concatenate([r["out"] for r in res.results], axis=0)


# revision 30
# speedup vs baseline: 1.7710x; 1.5473x over previous
"""Trainium2 Bass kernel for nn_ChannelMerger.

Computation (per batch b):
    emb   = fourier_emb(positions[b])            # [C, 288]
    scores= emb @ heads.T                        # [C, O] (transposed layout on device)
    w     = softmax(scores over C)
    out[b]= w.T @ meg[b]                         # [O, T]

Sharding: data-parallel over batch B=32 across 8 cores (4 batches/core).
heads (tiny) replicated. Everything computed on-device; host only reshapes
inputs (transpose positions/heads, constant table) and gathers outputs.

Device layout notes:
  - emb is built transposed ([d, c], d on partitions) so it can feed the
    scores matmul directly as the stationary operand.
  - softmax runs un-max-subtracted (scores are O(4), exp is safe in fp32);
    the 1/sum is folded into the PSUM->SBUF eviction of the PV matmul as a
    per-partition scale.
  - d-dimension is permuted (cos 0:128 | sin 0:128 | cos 128:144, sin
    128:144) so each ACT sin/cos call has matching in/out partition bases;
    heads rows are permuted identically on the host.
  - ACT's Sin is only valid on [-pi, pi]; arguments are range-reduced on DVE
    via an int32 cast (r = x - 2pi*int(x/2pi), one is_ge correction) since
    the HW has no mod/floor ALU op. The +pi/2 cos shift rides the loc matmul
    as a third contraction row.
"""

import math

import numpy as np

import concourse.bass as bass
import concourse.mybir as mybir
import concourse.tile as tile
from concourse import bacc

F32 = mybir.dt.float32
F16 = mybir.dt.float16  # single-pass PE matmul + FWL; fp32 is 2-pass/4x slower

B, C, T = 32, 273, 8192
O, D = 270, 288
N_CORES = 8
BPC = B // N_CORES  # batches per core
MARGIN = 0.2
N_FREQ = 12  # 12 freqs/axis; D = 2 * 12 * 12
TWO_PI = 2.0 * math.pi
HALF_PI = 0.5 * math.pi

TS = 2048  # T super-tile (per-DMA free size)
NSL = TS // 512  # 512-wide matmul slices per super-tile

C_CHUNKS = [(0, 128), (128, 128), (256, C - 256)]  # contraction over channels
O_CHUNKS = [(0, 128), (128, 128), (256, O - 256)]  # output-channel chunks
K_CHUNKS = [(0, 128), (128, 128), (256, 32)]  # device-d (permuted emb dim) chunks

# device-d row r <-> original emb dim perm[r] (see embT construction below)
PERM = (
    list(range(0, 128))  # cos(loc[0:128])
    + list(range(144, 272))  # sin(loc[0:128])
    + list(range(128, 144))  # cos(loc[128:144])
    + list(range(272, 288))  # sin(loc[128:144])
)

_SIN = mybir.ActivationFunctionType.Sin
_EXP = mybir.ActivationFunctionType.Exp


def _build_module() -> bass.Bass:
    # Bacc (not bare Bass): its compile() splits multi-sem waits — TRN2
    # instructions carry at most one wait condition and walrus rejects more.
    nc = bacc.Bacc()
    # meg/heads arrive as fp16 (host-cast): halves the dominant DMA read and
    # keeps every PE matmul single-pass at 1 cycle/row.
    meg_h = nc.dram_tensor("meg", [BPC, C, T], F16, kind="ExternalInput")
    posT_h = nc.dram_tensor("posT", [BPC, 3, C], F32, kind="ExternalInput")
    headsTp_h = nc.dram_tensor("headsTp", [D, O], F16, kind="ExternalInput")
    pconst_h = nc.dram_tensor("pconst", [3, D], F32, kind="ExternalInput")
    out_h = nc.dram_tensor("out", [BPC, O, T], F32, kind="ExternalOutput")

    with tile.TileContext(nc) as tc:
        with (
            tc.tile_pool(name="const", bufs=1) as const,
            tc.tile_pool(name="small", bufs=2) as small,
            tc.tile_pool(name="megp", bufs=4) as megp,
            tc.tile_pool(name="outp", bufs=4) as outp,
            tc.tile_pool(name="ps_small", bufs=1, space="PSUM") as ps_small,
            tc.tile_pool(name="ps_pv", bufs=4, space="PSUM") as ps_pv,
        ):
            # ---- persistent constants ----
            hT = []
            for ki, (k0, ksz) in enumerate(K_CHUNKS):
                t_ = const.tile([ksz, O], F16, tag=f"hT{ki}", name=f"hT{ki}")
                nc.sync.dma_start(out=t_, in_=headsTp_h[k0 : k0 + ksz, :])
                hT.append(t_)
            pconst_sb = const.tile([3, D], F32, tag="pconst", name="pconst_sb")
            nc.sync.dma_start(out=pconst_sb, in_=pconst_h[:, :])
            ones_c = const.tile([128, 1], F16, tag="ones", name="ones_c")
            nc.vector.memset(ones_c, 1.0)

            for b in range(BPC):
                # ---- fourier embedding (transposed: [d, c]) ----
                # posT rows: [x + margin, y + margin, 1]; pconst cols carry
                # (p_i, p_j, shift) so loc [+ pi/2 for cos] comes out of one
                # K=3 matmul per tile.
                posT_sb = small.tile([3, C], F32, tag="posT", name="posT_sb")
                nc.sync.dma_start(out=posT_sb, in_=posT_h[b])

                # pconst col r = (p_i, p_j, shift) for device-d row r, so each
                # K-chunk of the permuted embT comes from one matmul, one DVE
                # range-reduction, and one ACT Sin — all at partition base 0.
                embs = []
                for ki, (k0, ksz) in enumerate(K_CHUNKS):
                    loc_ps = ps_small.tile(
                        [128, C], F32, tag="loc", name="loc_ps", bufs=2
                    )[:ksz]
                    nc.tensor.matmul(
                        loc_ps,
                        pconst_sb[:, k0 : k0 + ksz],
                        posT_sb,
                        start=True,
                        stop=True,
                    )
                    # ACT Sin is valid on [-pi, pi] only and the HW has no
                    # mod/floor op: reduce with r = x - 2pi*int(x * 1/2pi)
                    # (x >= 0 here). Whether the int cast truncates or rounds
                    # to nearest, r lands in [-pi, 2pi); one is_ge(pi)
                    # correction of -2pi brings it into [-pi, pi).
                    n_i = small.tile(
                        [128, C], mybir.dt.int32, tag="n_i", name="n_i", bufs=3
                    )[:ksz]
                    nc.vector.tensor_scalar(
                        out=n_i,
                        in0=loc_ps,
                        scalar1=1.0 / TWO_PI,
                        scalar2=None,
                        op0=mybir.AluOpType.mult,
                    )
                    n_f = small.tile([128, C], F32, tag="n_f", name="n_f", bufs=3)[
                        :ksz
                    ]
                    nc.vector.tensor_copy(n_f, n_i)
                    nc.vector.tensor_scalar(
                        out=n_f,
                        in0=n_f,
                        scalar1=-TWO_PI,
                        scalar2=None,
                        op0=mybir.AluOpType.mult,
                    )
                    r_ = small.tile([128, C], F32, tag="r_", name="r_", bufs=3)[:ksz]
                    nc.vector.tensor_add(r_, loc_ps, n_f)
                    m_ = small.tile([128, C], F32, tag="m_", name="m_", bufs=3)[:ksz]
                    nc.vector.tensor_scalar(
                        out=m_,
                        in0=r_,
                        scalar1=math.pi,
                        scalar2=-TWO_PI,
                        op0=mybir.AluOpType.is_ge,
                        op1=mybir.AluOpType.mult,
                    )
                    nc.vector.tensor_add(r_, r_, m_)
                    e_ = small.tile([128, C], F16, tag=f"embT{ki}", name=f"embT{ki}")[
                        :ksz
                    ]
                    nc.scalar.activation(e_, r_, _SIN)
                    embs.append(e_)

                # ---- scores^T = embT.T-contract with headsTp, then exp ----
                expT = []
                for ci, (c0, csz) in enumerate(C_CHUNKS):
                    sc_ps = ps_small.tile([128, O], F32, tag="sc", name="sc_ps")[:csz]
                    for ki in range(3):
                        nc.tensor.matmul(
                            sc_ps,
                            embs[ki][:, c0 : c0 + csz],
                            hT[ki],
                            start=(ki == 0),
                            stop=(ki == 2),
                        )
                    e_ = small.tile([128, O], F16, tag=f"expT{ci}", name=f"expT{ci}")[
                        :csz
                    ]
                    nc.scalar.activation(e_, sc_ps, _EXP)
                    expT.append(e_)

                # ---- softmax denominators (per o), as per-partition vectors ----
                invs = []
                for oi, (o0, osz) in enumerate(O_CHUNKS):
                    sum_ps = ps_small.tile([128, 1], F32, tag="sum", name="sum_ps")[
                        :osz
                    ]
                    for ci, (c0, csz) in enumerate(C_CHUNKS):
                        nc.tensor.matmul(
                            sum_ps,
                            expT[ci][:, o0 : o0 + osz],
                            ones_c[:csz],
                            start=(ci == 0),
                            stop=(ci == 2),
                        )
                    iv = small.tile([128, 1], F32, tag=f"inv{oi}", name=f"inv{oi}")[
                        :osz
                    ]
                    nc.vector.reciprocal(iv, sum_ps)
                    invs.append(iv)

                # ---- PV: out[b, o, t] = invsum[o] * sum_c expT[c, o] meg[c, t] ----
                for ts in range(T // TS):
                    t0 = ts * TS
                    megs = []
                    for ci, (c0, csz) in enumerate(C_CHUNKS):
                        m_ = megp.tile([csz, TS], F16, tag=f"meg{ci}", name=f"meg{ci}")
                        nc.sync.dma_start(
                            out=m_, in_=meg_h[b, c0 : c0 + csz, t0 : t0 + TS]
                        )
                        megs.append(m_)
                    for oi, (o0, osz) in enumerate(O_CHUNKS):
                        ostage = outp.tile([128, TS], F32, tag="ostage", name="ostage")[
                            :osz
                        ]
                        for sl in range(NSL):
                            pv_ps = ps_pv.tile([128, 512], F32, tag="pv", name="pv_ps")[
                                :osz
                            ]
                            for ci in range(3):
                                nc.tensor.matmul(
                                    pv_ps,
                                    expT[ci][:, o0 : o0 + osz],
                                    megs[ci][:, sl * 512 : (sl + 1) * 512],
                                    start=(ci == 0),
                                    stop=(ci == 2),
                                )
                            dst = ostage[:, sl * 512 : (sl + 1) * 512]
                            # alternate eviction engine to split the load
                            if (oi * NSL + sl) % 2 == 0:
                                nc.scalar.mul(dst, pv_ps, mul=invs[oi])
                            else:
                                nc.vector.tensor_scalar_mul(dst, pv_ps, invs[oi])
                        nc.sync.dma_start(
                            out=out_h[b, o0 : o0 + osz, t0 : t0 + TS], in_=ostage
                        )
    nc.compile()
    return nc


_MODULE_CACHE: list = []


def _get_module() -> bass.Bass:
    if not _MODULE_CACHE:
        _MODULE_CACHE.append(_build_module())
    return _MODULE_CACHE[0]


def _host_prep(meg, positions, heads):
    """Shard + lay out inputs for the 8 cores."""
    freqs = (TWO_PI / (1.0 + 2.0 * MARGIN)) * np.arange(N_FREQ, dtype=np.float64)
    # pconst col r = (p_{L//12}, p_{L%12}, shift) for device-d row r, where
    # emb dim PERM[r] is cos(loc[L]) (shift pi/2) or sin(loc[L]) (shift 0).
    emb_dim = np.array(PERM)
    is_cos = emb_dim < 144
    L = np.where(is_cos, emb_dim, emb_dim - 144)
    pconst = np.stack(
        [freqs[L // N_FREQ], freqs[L % N_FREQ], np.where(is_cos, HALF_PI, 0.0)]
    ).astype(np.float32)  # [3, 288]

    headsTp = np.ascontiguousarray(heads[:, PERM].T).astype(np.float16)  # [288, 270]

    in_maps = []
    for k in range(N_CORES):
        sl = slice(k * BPC, (k + 1) * BPC)
        posT = np.concatenate(
            [
                positions[sl].transpose(0, 2, 1) + np.float32(MARGIN),
                np.ones((BPC, 1, C), dtype=np.float32),
            ],
            axis=1,
        )
        posT = np.ascontiguousarray(posT)
        in_maps.append(
            {
                "meg": np.ascontiguousarray(meg[sl]).astype(np.float16),
                "posT": posT,
                "headsTp": headsTp,
                "pconst": pconst,
            }
        )
    return in_maps


LAST_RESULTS = None  # BassKernelResults of the most recent kernel() call


def kernel(meg: np.ndarray, positions: np.ndarray, heads: np.ndarray) -> np.ndarray:
    global LAST_RESULTS
    from concourse.bass_utils import run_bass_kernel_spmd

    nc = _get_module()
    in_maps = _host_prep(
        np.asarray(meg, dtype=np.float32),
        np.asarray(positions, dtype=np.float32),
        np.asarray(heads, dtype=np.float32),
    )
    res = run_bass_kernel_spmd(nc, in_maps, core_ids=list(range(N_CORES)))
    LAST_RESULTS = res
    return np.concatenate([r["out"] for r in res.results], axis=0)


# revision 32
# speedup vs baseline: 2.5895x; 1.4622x over previous
"""Trainium2 Bass kernel for nn_ChannelMerger.

Computation (per batch b):
    emb   = fourier_emb(positions[b])            # [C, 288]
    scores= emb @ heads.T                        # [C, O] (transposed layout on device)
    w     = softmax(scores over C)
    out[b]= w.T @ meg[b]                         # [O, T]

Sharding: data-parallel over batch B=32 across 8 cores (4 batches/core).
heads (tiny) replicated. Everything computed on-device; host only reshapes
inputs (transpose positions/heads, constant table) and gathers outputs.

Device layout notes:
  - emb is built transposed ([d, c], d on partitions) so it can feed the
    scores matmul directly as the stationary operand.
  - softmax runs un-max-subtracted (scores are O(4), exp is safe in fp32);
    the 1/sum is folded into the PSUM->SBUF eviction of the PV matmul as a
    per-partition scale.
  - d-dimension is permuted (cos 0:128 | sin 0:128 | cos 128:144, sin
    128:144) so each ACT sin/cos call has matching in/out partition bases;
    heads rows are permuted identically on the host.
  - ACT's Sin is only valid on [-pi, pi]; arguments are range-reduced on DVE
    via an int32 cast (r = x - 2pi*int(x/2pi), one is_ge correction) since
    the HW has no mod/floor ALU op. The +pi/2 cos shift rides the loc matmul
    as a third contraction row.
"""

import math

import numpy as np

import concourse.bass as bass
import concourse.mybir as mybir
import concourse.tile as tile
from concourse import bacc

F32 = mybir.dt.float32
F16 = mybir.dt.float16  # single-pass PE matmul + FWL; fp32 is 2-pass/4x slower

B, C, T = 32, 273, 8192
O, D = 270, 288
N_CORES = 8
BPC = B // N_CORES  # batches per core
MARGIN = 0.2
N_FREQ = 12  # 12 freqs/axis; D = 2 * 12 * 12
TWO_PI = 2.0 * math.pi
HALF_PI = 0.5 * math.pi

TS = 2048  # T super-tile (per-DMA free size)
NSL = TS // 512  # 512-wide matmul slices per super-tile

C_CHUNKS = [(0, 128), (128, 128), (256, C - 256)]  # contraction over channels
O_CHUNKS = [(0, 128), (128, 128), (256, O - 256)]  # output-channel chunks
K_CHUNKS = [(0, 128), (128, 128), (256, 32)]  # device-d (permuted emb dim) chunks

# device-d row r <-> original emb dim perm[r] (see embT construction below)
PERM = (
    list(range(0, 128))  # cos(loc[0:128])
    + list(range(144, 272))  # sin(loc[0:128])
    + list(range(128, 144))  # cos(loc[128:144])
    + list(range(272, 288))  # sin(loc[128:144])
)

_SIN = mybir.ActivationFunctionType.Sin
_EXP = mybir.ActivationFunctionType.Exp


def _build_module() -> bass.Bass:
    # Bacc (not bare Bass): its compile() splits multi-sem waits — TRN2
    # instructions carry at most one wait condition and walrus rejects more.
    nc = bacc.Bacc()
    # meg/heads arrive as fp16 (host-cast): halves the dominant DMA read and
    # keeps every PE matmul single-pass at 1 cycle/row.
    meg_h = nc.dram_tensor("meg", [BPC, C, T], F16, kind="ExternalInput")
    posT_h = nc.dram_tensor("posT", [BPC, 3, C], F32, kind="ExternalInput")
    headsTp_h = nc.dram_tensor("headsTp", [D, O], F16, kind="ExternalInput")
    pconst_h = nc.dram_tensor("pconst", [3, D], F32, kind="ExternalInput")
    out_h = nc.dram_tensor("out", [BPC, O, T], F32, kind="ExternalOutput")

    with tile.TileContext(nc) as tc:
        with (
            tc.tile_pool(name="const", bufs=1) as const,
            tc.tile_pool(name="small", bufs=2) as small,
            tc.tile_pool(name="megp", bufs=4) as megp,
            tc.tile_pool(name="outp", bufs=4) as outp,
            # One PSUM tag: two rotating 4-bank slots. PV groups, loc, scores
            # and sums all share it, so the PE streams long uninterrupted MM
            # chains per slot (keeps the HAM clock-gate at full rate).
            tc.tile_pool(name="psum", bufs=2, space="PSUM") as psum,
        ):
            # ---- persistent constants ----
            hT = []
            for ki, (k0, ksz) in enumerate(K_CHUNKS):
                t_ = const.tile([ksz, O], F16, tag=f"hT{ki}", name=f"hT{ki}")
                nc.sync.dma_start(out=t_, in_=headsTp_h[k0 : k0 + ksz, :])
                hT.append(t_)
            pconst_sb = const.tile([3, D], F32, tag="pconst", name="pconst_sb")
            nc.sync.dma_start(out=pconst_sb, in_=pconst_h[:, :])
            ones_c = const.tile([128, 1], F16, tag="ones", name="ones_c")
            nc.vector.memset(ones_c, 1.0)

            for b in range(BPC):
                # ---- fourier embedding (transposed: [d, c]) ----
                # posT rows: [x + margin, y + margin, 1]; pconst cols carry
                # (p_i, p_j, shift) so loc [+ pi/2 for cos] comes out of one
                # K=3 matmul per tile.
                posT_sb = small.tile([3, C], F32, tag="posT", name="posT_sb")
                nc.sync.dma_start(out=posT_sb, in_=posT_h[b])

                # pconst col r = (p_i, p_j, shift) for device-d row r, so each
                # K-chunk of the permuted embT comes from one matmul, one DVE
                # range-reduction, and one ACT Sin — all at partition base 0.
                embs = []
                for ki, (k0, ksz) in enumerate(K_CHUNKS):
                    loc_ps = psum.tile([128, C], F32, tag="ps", name="loc_ps")[:ksz]
                    nc.tensor.matmul(
                        loc_ps,
                        pconst_sb[:, k0 : k0 + ksz],
                        posT_sb,
                        start=True,
                        stop=True,
                    )
                    # ACT Sin is valid on [-pi, pi] only and the HW has no
                    # mod/floor op: reduce with r = x - 2pi*int(x * 1/2pi)
                    # (x >= 0 here). Whether the int cast truncates or rounds
                    # to nearest, r lands in [-pi, 2pi); one is_ge(pi)
                    # correction of -2pi brings it into [-pi, pi).
                    n_i = small.tile(
                        [128, C], mybir.dt.int32, tag="n_i", name="n_i", bufs=3
                    )[:ksz]
                    nc.vector.tensor_scalar(
                        out=n_i,
                        in0=loc_ps,
                        scalar1=1.0 / TWO_PI,
                        scalar2=None,
                        op0=mybir.AluOpType.mult,
                    )
                    n_f = small.tile([128, C], F32, tag="n_f", name="n_f", bufs=3)[
                        :ksz
                    ]
                    nc.vector.tensor_copy(n_f, n_i)
                    nc.vector.tensor_scalar(
                        out=n_f,
                        in0=n_f,
                        scalar1=-TWO_PI,
                        scalar2=None,
                        op0=mybir.AluOpType.mult,
                    )
                    r_ = small.tile([128, C], F32, tag="r_", name="r_", bufs=3)[:ksz]
                    nc.vector.tensor_add(r_, loc_ps, n_f)
                    m_ = small.tile([128, C], F32, tag="m_", name="m_", bufs=3)[:ksz]
                    nc.vector.tensor_scalar(
                        out=m_,
                        in0=r_,
                        scalar1=math.pi,
                        scalar2=-TWO_PI,
                        op0=mybir.AluOpType.is_ge,
                        op1=mybir.AluOpType.mult,
                    )
                    nc.vector.tensor_add(r_, r_, m_)
                    e_ = small.tile([128, C], F16, tag=f"embT{ki}", name=f"embT{ki}")[
                        :ksz
                    ]
                    nc.scalar.activation(e_, r_, _SIN)
                    embs.append(e_)

                # ---- scores^T = embT.T-contract with headsTp, then exp ----
                expT = []
                for ci, (c0, csz) in enumerate(C_CHUNKS):
                    sc_ps = psum.tile([128, O], F32, tag="ps", name="sc_ps")[:csz]
                    for ki in range(3):
                        nc.tensor.matmul(
                            sc_ps,
                            embs[ki][:, c0 : c0 + csz],
                            hT[ki],
                            start=(ki == 0),
                            stop=(ki == 2),
                        )
                    e_ = small.tile([128, O], F16, tag=f"expT{ci}", name=f"expT{ci}")[
                        :csz
                    ]
                    nc.scalar.activation(e_, sc_ps, _EXP)
                    expT.append(e_)

                # ---- softmax denominators (per o), as per-partition vectors ----
                invs = []
                for oi, (o0, osz) in enumerate(O_CHUNKS):
                    sum_ps = psum.tile([128, 1], F32, tag="ps", name="sum_ps")[:osz]
                    for ci, (c0, csz) in enumerate(C_CHUNKS):
                        nc.tensor.matmul(
                            sum_ps,
                            expT[ci][:, o0 : o0 + osz],
                            ones_c[:csz],
                            start=(ci == 0),
                            stop=(ci == 2),
                        )
                    iv = small.tile([128, 1], F32, tag=f"inv{oi}", name=f"inv{oi}")[
                        :osz
                    ]
                    nc.vector.reciprocal(iv, sum_ps)
                    invs.append(iv)

                # ---- PV: out[b, o, t] = invsum[o] * sum_c expT[c, o] meg[c, t] ----
                for ts in range(T // TS):
                    t0 = ts * TS
                    megs = []
                    for ci, (c0, csz) in enumerate(C_CHUNKS):
                        m_ = megp.tile([csz, TS], F16, tag=f"meg{ci}", name=f"meg{ci}")
                        nc.sync.dma_start(
                            out=m_, in_=meg_h[b, c0 : c0 + csz, t0 : t0 + TS]
                        )
                        megs.append(m_)
                    for oi, (o0, osz) in enumerate(O_CHUNKS):
                        # 4-bank PSUM group; c outer / slice inner keeps the
                        # same weights resident for 4 back-to-back matmuls
                        pv_ps = psum.tile([128, TS], F32, tag="ps", name="pv_ps")[:osz]
                        for ci in range(3):
                            w_ = expT[ci][:, o0 : o0 + osz]
                            for sl in range(NSL):
                                nc.tensor.matmul(
                                    pv_ps[:, sl * 512 : (sl + 1) * 512],
                                    w_,
                                    megs[ci][:, sl * 512 : (sl + 1) * 512],
                                    start=(ci == 0),
                                    stop=(ci == 2),
                                )
                        ostage = outp.tile([128, TS], F32, tag="ostage", name="ostage")[
                            :osz
                        ]
                        # single whole-group eviction, engines alternating
                        if oi % 2 == 0:
                            nc.vector.tensor_scalar_mul(ostage, pv_ps, invs[oi])
                        else:
                            nc.scalar.mul(ostage, pv_ps, mul=invs[oi])
                        # stores ride the scalar-engine HWDGE queue so they
                        # never block the next loads on the sync queue
                        nc.scalar.dma_start(
                            out=out_h[b, o0 : o0 + osz, t0 : t0 + TS], in_=ostage
                        )
    nc.compile()
    return nc


_MODULE_CACHE: list = []


def _get_module() -> bass.Bass:
    if not _MODULE_CACHE:
        _MODULE_CACHE.append(_build_module())
    return _MODULE_CACHE[0]


def _host_prep(meg, positions, heads):
    """Shard + lay out inputs for the 8 cores."""
    freqs = (TWO_PI / (1.0 + 2.0 * MARGIN)) * np.arange(N_FREQ, dtype=np.float64)
    # pconst col r = (p_{L//12}, p_{L%12}, shift) for device-d row r, where
    # emb dim PERM[r] is cos(loc[L]) (shift pi/2) or sin(loc[L]) (shift 0).
    emb_dim = np.array(PERM)
    is_cos = emb_dim < 144
    L = np.where(is_cos, emb_dim, emb_dim - 144)
    pconst = np.stack(
        [freqs[L // N_FREQ], freqs[L % N_FREQ], np.where(is_cos, HALF_PI, 0.0)]
    ).astype(np.float32)  # [3, 288]

    headsTp = np.ascontiguousarray(heads[:, PERM].T).astype(np.float16)  # [288, 270]

    in_maps = []
    for k in range(N_CORES):
        sl = slice(k * BPC, (k + 1) * BPC)
        posT = np.concatenate(
            [
                positions[sl].transpose(0, 2, 1) + np.float32(MARGIN),
                np.ones((BPC, 1, C), dtype=np.float32),
            ],
            axis=1,
        )
        posT = np.ascontiguousarray(posT)
        in_maps.append(
            {
                "meg": np.ascontiguousarray(meg[sl]).astype(np.float16),
                "posT": posT,
                "headsTp": headsTp,
                "pconst": pconst,
            }
        )
    return in_maps


LAST_RESULTS = None  # BassKernelResults of the most recent kernel() call


def kernel(meg: np.ndarray, positions: np.ndarray, heads: np.ndarray) -> np.ndarray:
    global LAST_RESULTS
    from concourse.bass_utils import run_bass_kernel_spmd

    nc = _get_module()
    in_maps = _host_prep(
        np.asarray(meg, dtype=np.float32),
        np.asarray(positions, dtype=np.float32),
        np.asarray(heads, dtype=np.float32),
    )
    res = run_bass_kernel_spmd(nc, in_maps, core_ids=list(range(N_CORES)))
    LAST_RESULTS = res
    return np.concatenate([r["out"] for r in res.results], axis=0)


# revision 33
# speedup vs baseline: 2.7823x; 1.0744x over previous
"""Trainium2 Bass kernel for nn_ChannelMerger.

Computation (per batch b):
    emb   = fourier_emb(positions[b])            # [C, 288]
    scores= emb @ heads.T                        # [C, O] (transposed layout on device)
    w     = softmax(scores over C)
    out[b]= w.T @ meg[b]                         # [O, T]

Sharding: data-parallel over batch B=32 across 8 cores (4 batches/core).
heads (tiny) replicated. Everything computed on-device; host only reshapes
inputs (transpose positions/heads, constant table) and gathers outputs.

Device layout notes:
  - emb is built transposed ([d, c], d on partitions) so it can feed the
    scores matmul directly as the stationary operand.
  - softmax runs un-max-subtracted (scores are O(4), exp is safe in fp32);
    the 1/sum is folded into the PSUM->SBUF eviction of the PV matmul as a
    per-partition scale.
  - d-dimension is permuted (cos 0:128 | sin 0:128 | cos 128:144, sin
    128:144) so each ACT sin/cos call has matching in/out partition bases;
    heads rows are permuted identically on the host.
  - ACT's Sin is only valid on [-pi, pi]; arguments are range-reduced on DVE
    via an int32 cast (r = x - 2pi*int(x/2pi), one is_ge correction) since
    the HW has no mod/floor ALU op. The +pi/2 cos shift rides the loc matmul
    as a third contraction row.
"""

import math

import numpy as np

import concourse.bass as bass
import concourse.mybir as mybir
import concourse.tile as tile
from concourse import bacc

F32 = mybir.dt.float32
F16 = mybir.dt.float16  # single-pass PE matmul + FWL; fp32 is 2-pass/4x slower

B, C, T = 32, 273, 8192
O, D = 270, 288
N_CORES = 8
BPC = B // N_CORES  # batches per core
MARGIN = 0.2
N_FREQ = 12  # 12 freqs/axis; D = 2 * 12 * 12
TWO_PI = 2.0 * math.pi
HALF_PI = 0.5 * math.pi

TS = 2048  # T super-tile (per-DMA free size)
NSL = TS // 512  # 512-wide matmul slices per super-tile

C_CHUNKS = [(0, 128), (128, 128), (256, C - 256)]  # contraction over channels
O_CHUNKS = [(0, 128), (128, 128), (256, O - 256)]  # output-channel chunks
K_CHUNKS = [(0, 128), (128, 128), (256, 32)]  # device-d (permuted emb dim) chunks

# device-d row r <-> original emb dim perm[r] (see embT construction below)
PERM = (
    list(range(0, 128))  # cos(loc[0:128])
    + list(range(144, 272))  # sin(loc[0:128])
    + list(range(128, 144))  # cos(loc[128:144])
    + list(range(272, 288))  # sin(loc[128:144])
)

_SIN = mybir.ActivationFunctionType.Sin
_EXP = mybir.ActivationFunctionType.Exp


def _build_module() -> bass.Bass:
    # Bacc (not bare Bass): its compile() splits multi-sem waits — TRN2
    # instructions carry at most one wait condition and walrus rejects more.
    nc = bacc.Bacc()
    # meg/heads arrive as fp16 (host-cast): halves the dominant DMA read and
    # keeps every PE matmul single-pass at 1 cycle/row.
    meg_h = nc.dram_tensor("meg", [BPC, C, T], F16, kind="ExternalInput")
    posT_h = nc.dram_tensor("posT", [BPC, 3, C], F32, kind="ExternalInput")
    headsTp_h = nc.dram_tensor("headsTp", [D, O], F16, kind="ExternalInput")
    pconst_h = nc.dram_tensor("pconst", [3, D], F32, kind="ExternalInput")
    out_h = nc.dram_tensor("out", [BPC, O, T], F32, kind="ExternalOutput")

    with tile.TileContext(nc) as tc:
        with (
            tc.tile_pool(name="const", bufs=1) as const,
            tc.tile_pool(name="small", bufs=2) as small,
            tc.tile_pool(name="megp", bufs=4) as megp,
            tc.tile_pool(name="outp", bufs=4) as outp,
            # One PSUM tag: two rotating 4-bank slots. PV groups, loc, scores
            # and sums all share it, so the PE streams long uninterrupted MM
            # chains per slot (keeps the HAM clock-gate at full rate).
            tc.tile_pool(name="psum", bufs=2, space="PSUM") as psum,
        ):
            # ---- persistent constants ----
            hT = []
            for ki, (k0, ksz) in enumerate(K_CHUNKS):
                t_ = const.tile([ksz, O], F16, tag=f"hT{ki}", name=f"hT{ki}")
                nc.sync.dma_start(out=t_, in_=headsTp_h[k0 : k0 + ksz, :])
                hT.append(t_)
            pconst_sb = const.tile([3, D], F32, tag="pconst", name="pconst_sb")
            nc.sync.dma_start(out=pconst_sb, in_=pconst_h[:, :])
            ones_c = const.tile([128, 1], F16, tag="ones", name="ones_c")
            nc.vector.memset(ones_c, 1.0)

            # ---- phase 1: softmax weights for ALL batches up front, so the
            # PV phase below is one dense uninterrupted PE stream (no
            # mid-kernel stalls, HAM clock-gate stays warm) ----
            expT_all = []
            invs_all = []
            for b in range(BPC):
                # ---- fourier embedding (transposed: [d, c]) ----
                # posT rows: [x + margin, y + margin, 1]; pconst cols carry
                # (p_i, p_j, shift) so loc [+ pi/2 for cos] comes out of one
                # K=3 matmul per tile.
                posT_sb = small.tile([3, C], F32, tag="posT", name="posT_sb", bufs=4)
                nc.sync.dma_start(out=posT_sb, in_=posT_h[b])

                # pconst col r = (p_i, p_j, shift) for device-d row r, so each
                # K-chunk of the permuted embT comes from one matmul, one DVE
                # range-reduction, and one ACT Sin — all at partition base 0.
                embs = []
                for ki, (k0, ksz) in enumerate(K_CHUNKS):
                    loc_ps = psum.tile([128, C], F32, tag="ps", name="loc_ps")[:ksz]
                    nc.tensor.matmul(
                        loc_ps,
                        pconst_sb[:, k0 : k0 + ksz],
                        posT_sb,
                        start=True,
                        stop=True,
                    )
                    # ACT Sin is valid on [-pi, pi] only and the HW has no
                    # mod/floor op: reduce with r = x - 2pi*int(x * 1/2pi)
                    # (x >= 0 here). Whether the int cast truncates or rounds
                    # to nearest, r lands in [-pi, 2pi); one is_ge(pi)
                    # correction of -2pi brings it into [-pi, pi).
                    n_i = small.tile(
                        [128, C], mybir.dt.int32, tag="n_i", name="n_i", bufs=3
                    )[:ksz]
                    nc.vector.tensor_scalar(
                        out=n_i,
                        in0=loc_ps,
                        scalar1=1.0 / TWO_PI,
                        scalar2=None,
                        op0=mybir.AluOpType.mult,
                    )
                    n_f = small.tile([128, C], F32, tag="n_f", name="n_f", bufs=3)[
                        :ksz
                    ]
                    nc.vector.tensor_copy(n_f, n_i)
                    nc.vector.tensor_scalar(
                        out=n_f,
                        in0=n_f,
                        scalar1=-TWO_PI,
                        scalar2=None,
                        op0=mybir.AluOpType.mult,
                    )
                    r_ = small.tile([128, C], F32, tag="r_", name="r_", bufs=3)[:ksz]
                    nc.vector.tensor_add(r_, loc_ps, n_f)
                    m_ = small.tile([128, C], F32, tag="m_", name="m_", bufs=3)[:ksz]
                    nc.vector.tensor_scalar(
                        out=m_,
                        in0=r_,
                        scalar1=math.pi,
                        scalar2=-TWO_PI,
                        op0=mybir.AluOpType.is_ge,
                        op1=mybir.AluOpType.mult,
                    )
                    nc.vector.tensor_add(r_, r_, m_)
                    e_ = small.tile([128, C], F16, tag=f"embT{ki}", name=f"embT{ki}")[
                        :ksz
                    ]
                    nc.scalar.activation(e_, r_, _SIN)
                    embs.append(e_)

                # ---- scores^T = embT.T-contract with headsTp, then exp ----
                expT = []
                for ci, (c0, csz) in enumerate(C_CHUNKS):
                    sc_ps = psum.tile([128, O], F32, tag="ps", name="sc_ps")[:csz]
                    for ki in range(3):
                        nc.tensor.matmul(
                            sc_ps,
                            embs[ki][:, c0 : c0 + csz],
                            hT[ki],
                            start=(ki == 0),
                            stop=(ki == 2),
                        )
                    e_ = small.tile(
                        [128, O], F16, tag=f"expT{ci}", name=f"expT{ci}", bufs=4
                    )[:csz]
                    nc.scalar.activation(e_, sc_ps, _EXP)
                    expT.append(e_)

                # ---- softmax denominators (per o), as per-partition vectors ----
                invs = []
                for oi, (o0, osz) in enumerate(O_CHUNKS):
                    sum_ps = psum.tile([128, 1], F32, tag="ps", name="sum_ps")[:osz]
                    for ci, (c0, csz) in enumerate(C_CHUNKS):
                        nc.tensor.matmul(
                            sum_ps,
                            expT[ci][:, o0 : o0 + osz],
                            ones_c[:csz],
                            start=(ci == 0),
                            stop=(ci == 2),
                        )
                    iv = small.tile(
                        [128, 1], F32, tag=f"inv{oi}", name=f"inv{oi}", bufs=4
                    )[:osz]
                    nc.vector.reciprocal(iv, sum_ps)
                    invs.append(iv)

                expT_all.append(expT)
                invs_all.append(invs)

            # ---- phase 2: PV for all batches, one dense PE stream ----
            # out[b, o, t] = invsum[o] * sum_c expT[c, o] meg[c, t]
            for b in range(BPC):
                expT = expT_all[b]
                invs = invs_all[b]
                for ts in range(T // TS):
                    t0 = ts * TS
                    megs = []
                    for ci, (c0, csz) in enumerate(C_CHUNKS):
                        m_ = megp.tile([csz, TS], F16, tag=f"meg{ci}", name=f"meg{ci}")
                        nc.sync.dma_start(
                            out=m_, in_=meg_h[b, c0 : c0 + csz, t0 : t0 + TS]
                        )
                        megs.append(m_)
                    for oi, (o0, osz) in enumerate(O_CHUNKS):
                        # 4-bank PSUM group; c outer / slice inner keeps the
                        # same weights resident for 4 back-to-back matmuls
                        pv_ps = psum.tile([128, TS], F32, tag="ps", name="pv_ps")[:osz]
                        for ci in range(3):
                            w_ = expT[ci][:, o0 : o0 + osz]
                            for sl in range(NSL):
                                nc.tensor.matmul(
                                    pv_ps[:, sl * 512 : (sl + 1) * 512],
                                    w_,
                                    megs[ci][:, sl * 512 : (sl + 1) * 512],
                                    start=(ci == 0),
                                    stop=(ci == 2),
                                )
                        ostage = outp.tile([128, TS], F32, tag="ostage", name="ostage")[
                            :osz
                        ]
                        # eviction split across both engines so the PSUM
                        # slot frees in ~1us
                        half = TS // 2
                        nc.vector.tensor_scalar_mul(
                            ostage[:, 0:half], pv_ps[:, 0:half], invs[oi]
                        )
                        nc.scalar.mul(
                            ostage[:, half:TS], pv_ps[:, half:TS], mul=invs[oi]
                        )
                        # stores ride the scalar-engine HWDGE queue so they
                        # never block the next loads on the sync queue
                        nc.scalar.dma_start(
                            out=out_h[b, o0 : o0 + osz, t0 : t0 + TS], in_=ostage
                        )
    nc.compile()
    return nc


_MODULE_CACHE: list = []


def _get_module() -> bass.Bass:
    if not _MODULE_CACHE:
        _MODULE_CACHE.append(_build_module())
    return _MODULE_CACHE[0]


def _host_prep(meg, positions, heads):
    """Shard + lay out inputs for the 8 cores."""
    freqs = (TWO_PI / (1.0 + 2.0 * MARGIN)) * np.arange(N_FREQ, dtype=np.float64)
    # pconst col r = (p_{L//12}, p_{L%12}, shift) for device-d row r, where
    # emb dim PERM[r] is cos(loc[L]) (shift pi/2) or sin(loc[L]) (shift 0).
    emb_dim = np.array(PERM)
    is_cos = emb_dim < 144
    L = np.where(is_cos, emb_dim, emb_dim - 144)
    pconst = np.stack(
        [freqs[L // N_FREQ], freqs[L % N_FREQ], np.where(is_cos, HALF_PI, 0.0)]
    ).astype(np.float32)  # [3, 288]

    headsTp = np.ascontiguousarray(heads[:, PERM].T).astype(np.float16)  # [288, 270]

    in_maps = []
    for k in range(N_CORES):
        sl = slice(k * BPC, (k + 1) * BPC)
        posT = np.concatenate(
            [
                positions[sl].transpose(0, 2, 1) + np.float32(MARGIN),
                np.ones((BPC, 1, C), dtype=np.float32),
            ],
            axis=1,
        )
        posT = np.ascontiguousarray(posT)
        in_maps.append(
            {
                "meg": np.ascontiguousarray(meg[sl]).astype(np.float16),
                "posT": posT,
                "headsTp": headsTp,
                "pconst": pconst,
            }
        )
    return in_maps


LAST_RESULTS = None  # BassKernelResults of the most recent kernel() call


def kernel(meg: np.ndarray, positions: np.ndarray, heads: np.ndarray) -> np.ndarray:
    global LAST_RESULTS
    from concourse.bass_utils import run_bass_kernel_spmd

    nc = _get_module()
    in_maps = _host_prep(
        np.asarray(meg, dtype=np.float32),
        np.asarray(positions, dtype=np.float32),
        np.asarray(heads, dtype=np.float32),
    )
    res = run_bass_kernel_spmd(nc, in_maps, core_ids=list(range(N_CORES)))
    LAST_RESULTS = res
    return np.concatenate([r["out"] for r in res.results], axis=0)
